# revision 24
# baseline (speedup 1.0000x reference)
"""TRN2 Bass kernel for nn_Attention_m_17815524344494.

Multi-head attention over [B=8, M=4, P=512, H=768], nh=12, hs=64.
Sharding: data-parallel over batch B -> one batch element per NeuronCore (8 cores).

Per-core dataflow (T = M*P = 2048 tokens, all matmuls in float32r ~ TF32):
  1. x [2048,768] -> PE-transpose -> xT (feature-major) per modality
  2. qT = Wq^T xT, kT = Wk^T xT (feature-major), v = x Wv (token-major,
     augmented with a ones column per head for free softmax sums)
  3. per (modality, head): scoresT = kT^T q (keys on partitions),
     eT = exp(scoresT/8) via ScalarE, ctxT_unnorm/sums = v_aug^T eT,
     sums broadcast via K=1 outer-product matmul, normalize on VectorE
  4. out = ctxT^T Wo (token-major), DMA to DRAM

Biases are zeros per the problem spec; a numpy fallback handles the
(never exercised) nonzero-bias case.
"""

from contextlib import ExitStack

import numpy as np

import concourse.bass as bass
import concourse.mybir as mybir
from concourse import bacc, bass_utils
from concourse.tile import TileContext
from concourse.masks import make_identity

F32 = mybir.dt.float32
F32R = mybir.dt.float32r
AF = mybir.ActivationFunctionType
ALU = mybir.AluOpType

B, M, PM, H = 8, 4, 512, 768
NH, HS = 12, 64
T = M * PM          # 2048 tokens per core
HC = H // 128       # 6 hidden chunks
TCM = PM // 128     # 4 token chunks per modality


def _emit(tc, ctx):
    nc = tc.nc

    x_ap = nc.dram_tensor("x", [T, H], F32, kind="ExternalInput").ap()
    wq_ap = nc.dram_tensor("wq", [H, H], F32, kind="ExternalInput").ap()
    wk_ap = nc.dram_tensor("wk", [H, H], F32, kind="ExternalInput").ap()
    wv_ap = nc.dram_tensor("wv", [H, H], F32, kind="ExternalInput").ap()
    wo_ap = nc.dram_tensor("wo", [H, H], F32, kind="ExternalInput").ap()
    out_ap = nc.dram_tensor("out", [T, H], F32, kind="ExternalOutput").ap()

    const = ctx.enter_context(tc.tile_pool(name="const", bufs=1))

    # f32r tiles can't be written by memset/affine_select directly (no
    # f32r rounding on those ISA paths); stage in f32 and copy via DVE.
    identity = const.tile([128, 128], F32R)
    onescol = const.tile([128, NH * TCM], F32R)
    # Selector for broadcasting softmax-sum reciprocals: sel[k, h*64+c] = (k==h),
    # so sel[:, h*64:(h+1)*64].T @ rs12 replicates row h onto 64 partitions.
    sel = const.tile([NH, NH * 64], F32R)
    with tc.tile_pool(name="stage", bufs=1) as stage:
        id_stage = stage.tile([128, 128], F32)
        make_identity(nc, id_stage[:])
        nc.vector.tensor_copy(identity[:], id_stage[:])
        ones_stage = stage.tile([128, 64], F32)
        nc.gpsimd.memset(ones_stage[:], 1.0)
        nc.vector.tensor_copy(onescol[:], ones_stage[:, :NH * TCM])
        selc = stage.tile([NH, NH * 64], F32)
        nc.gpsimd.memset(selc[:], 1.0)
        nc.gpsimd.affine_select(
            out=selc[:], in_=selc[:], compare_op=ALU.is_ge, fill=0.0,
            base=0, pattern=[[1, NH * 64]], channel_multiplier=-64,
        )
        nc.gpsimd.affine_select(
            out=selc[:], in_=selc[:], compare_op=ALU.is_ge, fill=0.0,
            base=63, pattern=[[-1, NH * 64]], channel_multiplier=64,
        )
        nc.vector.tensor_copy(sel[:], selc[:])

    wpool = ctx.enter_context(tc.tile_pool(name="w", bufs=1))
    xin = ctx.enter_context(tc.tile_pool(name="xin", bufs=2))
    xtp = ctx.enter_context(tc.tile_pool(name="xt", bufs=1))
    qpool = ctx.enter_context(tc.tile_pool(name="q", bufs=2))
    kpool = ctx.enter_context(tc.tile_pool(name="k", bufs=2))
    vpool = ctx.enter_context(tc.tile_pool(name="v", bufs=2))
    epool = ctx.enter_context(tc.tile_pool(name="e", bufs=2))
    spool = ctx.enter_context(tc.tile_pool(name="s", bufs=1))
    stpool = ctx.enter_context(tc.tile_pool(name="st", bufs=2))
    rpool = ctx.enter_context(tc.tile_pool(name="r", bufs=1))
    cpool = ctx.enter_context(tc.tile_pool(name="ctx", bufs=1))
    opool = ctx.enter_context(tc.tile_pool(name="o", bufs=1))
    ps_big = ctx.enter_context(tc.tile_pool(name="ps_big", bufs=2, space="PSUM"))
    ps_sc = ctx.enter_context(tc.tile_pool(name="ps_sc", bufs=1, space="PSUM"))
    ps_c = ctx.enter_context(tc.tile_pool(name="ps_c", bufs=2, space="PSUM"))

    w_tiles = {}
    for name, ap in (("wq", wq_ap), ("wk", wk_ap), ("wv", wv_ap), ("wo", wo_ap)):
        t = wpool.tile([128, HC, H], F32R, tag=name)
        nc.gpsimd.dma_start(t[:], ap.rearrange("(kc p) j -> p kc j", p=128))
        w_tiles[name] = t

    for m in range(M):
        # ---- Phase A: load x tiles for this modality, transpose to xT ----
        xt = xtp.tile([128, HC, PM], F32R, tag="xt")
        for ti in range(TCM):
            xtile = xin.tile([128, H], F32R, tag="xin")
            row0 = (m * TCM + ti) * 128
            nc.gpsimd.dma_start(xtile[:], x_ap[row0:row0 + 128, :])
            for hc in range(HC):
                pst = ps_big.tile([128, 512], F32R, tag="ps_big")
                nc.tensor.transpose(
                    pst[:, :128], xtile[:, hc * 128:(hc + 1) * 128], identity[:]
                )
                nc.vector.tensor_copy(
                    xt[:, hc, ti * 128:(ti + 1) * 128], pst[:, :128]
                )

        # ---- Phase B: projections ----
        qt = qpool.tile([128, HC, PM], F32R, tag="q")
        kt = kpool.tile([128, HC, PM], F32R, tag="k")
        for dst, w in ((qt, w_tiles["wq"]), (kt, w_tiles["wk"])):
            for jc in range(HC):
                ps = ps_big.tile([128, 512], F32, tag="ps_big")
                for kc in range(HC):
                    nc.tensor.matmul(
                        ps[:],
                        w[:, kc, jc * 128:(jc + 1) * 128],
                        xt[:, kc, :],
                        start=(kc == 0),
                        stop=(kc == HC - 1),
                    )
                nc.vector.tensor_copy(dst[:, jc, :], ps[:])

        vt = vpool.tile([128, TCM, NH, HS + 1], F32R, tag="v")
        nc.vector.tensor_copy(
            vt[:, :, :, HS],
            onescol[:].rearrange("p (t h) -> p t h", t=TCM),
        )
        for ti in range(TCM):
            for nn in range(2):
                ps = ps_big.tile([128, 512], F32, tag="ps_big")
                for kc in range(HC):
                    nc.tensor.matmul(
                        ps[:, :384],
                        xt[:, kc, ti * 128:(ti + 1) * 128],
                        w_tiles["wv"][:, kc, nn * 384:(nn + 1) * 384],
                        start=(kc == 0),
                        stop=(kc == HC - 1),
                    )
                nc.vector.tensor_copy(
                    vt[:, ti, nn * 6:(nn + 1) * 6, :HS],
                    ps[:, :384].rearrange("p (h c) -> p h c", c=HS),
                )

        # ---- Phase C: attention per head ----
        # Unnormalized ctx goes into ctxt; per-head softmax sums are
        # gathered into srows, reciprocal'd in ONE batched DVE op
        # (reciprocal duration is set by the free dim, so [12,512] costs
        # the same as [1,512]), then normalization is applied in-place.
        ctxt = cpool.tile([128, HC, PM], F32R, tag="ctx")
        srows = spool.tile([NH, 512], F32, tag="s")
        for h in range(NH):
            hc, hr = h // 2, (h % 2) * 64
            qh = qt[hr:hr + 64, hc, :]
            pssc = ps_sc.tile([128, TCM * 512], F32, tag="ps_sc")
            for jc in range(TCM):
                nc.tensor.matmul(
                    pssc[:, jc * 512:(jc + 1) * 512],
                    kt[hr:hr + 64, hc, jc * 128:(jc + 1) * 128],
                    qh,
                    start=True,
                    stop=True,
                )
            et = epool.tile([128, TCM * 512], F32R, tag="e")
            nc.scalar.activation(et[:], pssc[:], AF.Exp, scale=0.125)
            psc = ps_c.tile([HS + 1, 512], F32, tag="ps_c")
            for jc in range(TCM):
                nc.tensor.matmul(
                    psc[:],
                    vt[:, jc, h, :],
                    et[:, jc * 512:(jc + 1) * 512],
                    start=(jc == 0),
                    stop=(jc == TCM - 1),
                )
            nc.vector.tensor_copy(ctxt[hr:hr + 64, hc, :], psc[:HS, :])
            stmp = stpool.tile([1, 512], F32, tag="stmp")
            nc.vector.tensor_copy(stmp[:], psc[HS:HS + 1, :])
            nc.sync.dma_start(srows[h:h + 1, :], stmp[:])
        rs12 = rpool.tile([NH, 512], F32R, tag="r")
        with nc.allow_low_precision(reason="softmax recip in f32r is plenty"):
            nc.vector.reciprocal(rs12[:], srows[:])
        for h in range(NH):
            hc, hr = h // 2, (h % 2) * 64
            psb = ps_big.tile([128, 512], F32, tag="ps_big")
            nc.tensor.matmul(
                psb[:64, :], sel[:, h * 64:(h + 1) * 64], rs12[:],
                start=True, stop=True,
            )
            nc.vector.tensor_tensor(
                ctxt[hr:hr + 64, hc, :], ctxt[hr:hr + 64, hc, :],
                psb[:64, :], ALU.mult,
            )

        # ---- Phase D: output projection ----
        for ti in range(TCM):
            osb = opool.tile([128, H], F32, tag="o")
            for nn in range(2):
                ps = ps_big.tile([128, 512], F32, tag="ps_big")
                for cc in range(HC):
                    nc.tensor.matmul(
                        ps[:, :384],
                        ctxt[:, cc, ti * 128:(ti + 1) * 128],
                        w_tiles["wo"][:, cc, nn * 384:(nn + 1) * 384],
                        start=(cc == 0),
                        stop=(cc == HC - 1),
                    )
                nc.vector.tensor_copy(osb[:, nn * 384:(nn + 1) * 384], ps[:, :384])
            row0 = (m * TCM + ti) * 128
            nc.sync.dma_start(out_ap[row0:row0 + 128, :], osb[:])


_NC_CACHE = {}


def build_nc():
    if "nc" not in _NC_CACHE:
        nc = bacc.Bacc("TRN2", target_bir_lowering=False, debug=False, num_devices=B)
        with TileContext(nc) as tc:
            with ExitStack() as stack:
                _emit(tc, stack)
        nc.compile()
        _NC_CACHE["nc"] = nc
    return _NC_CACHE["nc"]


def _numpy_fallback(x, Wq, bq, Wk, bk, Wv, bv, Wo, bo):
    Bb, Mm, Pp, Hh = x.shape
    xx = x.reshape(-1, Hh)
    q = (xx @ Wq + bq).reshape(Bb, Mm, Pp, NH, HS).transpose(0, 1, 3, 2, 4)
    k = (xx @ Wk + bk).reshape(Bb, Mm, Pp, NH, HS).transpose(0, 1, 3, 2, 4)
    v = (xx @ Wv + bv).reshape(Bb, Mm, Pp, NH, HS).transpose(0, 1, 3, 2, 4)
    s = np.einsum("bmnqh,bmnkh->bmnqk", q, k) / np.sqrt(HS)
    s = s - s.max(axis=-1, keepdims=True)
    e = np.exp(s)
    p = e / e.sum(axis=-1, keepdims=True)
    ctx = np.einsum("bmnqk,bmnkh->bmnqh", p, v)
    ctx = ctx.transpose(0, 1, 3, 2, 4).reshape(Bb, Mm, Pp, Hh)
    return (ctx @ Wo + bo).astype(np.float32)


def kernel(hidden_states, Wq, bq, Wk, bk, Wv, bv, Wo, bo):
    hs = np.ascontiguousarray(np.asarray(hidden_states, dtype=np.float32))
    ws = {n: np.ascontiguousarray(np.asarray(w, dtype=np.float32))
          for n, w in (("wq", Wq), ("wk", Wk), ("wv", Wv), ("wo", Wo))}
    biases = [np.asarray(b, dtype=np.float32) for b in (bq, bk, bv, bo)]
    if any(np.any(b) for b in biases):
        return _numpy_fallback(hs, ws["wq"], biases[0], ws["wk"], biases[1],
                               ws["wv"], biases[2], ws["wo"], biases[3])

    nc = build_nc()
    in_maps = [
        {"x": hs[b].reshape(T, H), **ws}
        for b in range(B)
    ]
    res = bass_utils.run_bass_kernel_spmd(nc, in_maps, core_ids=list(range(B)))
    out = np.stack([res.results[b]["out"].reshape(M, PM, H) for b in range(B)])
    return out.astype(np.float32)


# revision 26
# speedup vs baseline: 1.4227x; 1.4227x over previous
"""TRN2 Bass kernel for nn_Attention_m_17815524344494.

Multi-head attention over [B=8, M=4, P=512, H=768], nh=12, hs=64.
Sharding: data-parallel over batch B -> one batch element per NeuronCore (8 cores).

Per-core dataflow (T = M*P = 2048 tokens, all matmuls in float32r ~ TF32):
  1. x [2048,768] -> PE-transpose -> xT (feature-major) per modality
  2. qT = Wq^T xT, kT = Wk^T xT (feature-major), v = x Wv (token-major,
     augmented with a ones column per head for free softmax sums)
  3. per (modality, head): scoresT = kT^T q (keys on partitions),
     eT = exp(scoresT/8) via ScalarE, ctxT_unnorm/sums = v_aug^T eT,
     sums broadcast via K=1 outer-product matmul, normalize on VectorE
  4. out = ctxT^T Wo (token-major), DMA to DRAM

Biases are zeros per the problem spec; a numpy fallback handles the
(never exercised) nonzero-bias case.
"""

from contextlib import ExitStack

import numpy as np

import concourse.bass as bass
import concourse.mybir as mybir
from concourse import bacc, bass_utils
from concourse.tile import TileContext
from concourse.masks import make_identity

F32 = mybir.dt.float32
F32R = mybir.dt.float32r
AF = mybir.ActivationFunctionType
ALU = mybir.AluOpType

B, M, PM, H = 8, 4, 512, 768
NH, HS = 12, 64
T = M * PM          # 2048 tokens per core
HC = H // 128       # 6 hidden chunks
TCM = PM // 128     # 4 token chunks per modality


def _emit(tc, ctx):
    nc = tc.nc

    x_ap = nc.dram_tensor("x", [T, H], F32, kind="ExternalInput").ap()
    wq_ap = nc.dram_tensor("wq", [H, H], F32, kind="ExternalInput").ap()
    wk_ap = nc.dram_tensor("wk", [H, H], F32, kind="ExternalInput").ap()
    wv_ap = nc.dram_tensor("wv", [H, H], F32, kind="ExternalInput").ap()
    wo_ap = nc.dram_tensor("wo", [H, H], F32, kind="ExternalInput").ap()
    out_ap = nc.dram_tensor("out", [T, H], F32, kind="ExternalOutput").ap()

    const = ctx.enter_context(tc.tile_pool(name="const", bufs=1))

    # f32r tiles can't be written by memset/affine_select directly (no
    # f32r rounding on those ISA paths); stage in f32 and copy via DVE.
    identity = const.tile([128, 128], F32R)
    onescol = const.tile([128, NH * TCM], F32R)
    ones64 = const.tile([1, 64], F32R)
    with tc.tile_pool(name="stage", bufs=1) as stage:
        id_stage = stage.tile([128, 128], F32)
        make_identity(nc, id_stage[:])
        nc.vector.tensor_copy(identity[:], id_stage[:])
        ones_stage = stage.tile([128, 64], F32)
        nc.gpsimd.memset(ones_stage[:], 1.0)
        nc.vector.tensor_copy(onescol[:], ones_stage[:, :NH * TCM])
        nc.vector.tensor_copy(ones64[:], ones_stage[:1, :])

    wpool = ctx.enter_context(tc.tile_pool(name="w", bufs=1))
    xin = ctx.enter_context(tc.tile_pool(name="xin", bufs=2))
    xtp = ctx.enter_context(tc.tile_pool(name="xt", bufs=1))
    qpool = ctx.enter_context(tc.tile_pool(name="q", bufs=2))
    kpool = ctx.enter_context(tc.tile_pool(name="k", bufs=2))
    vpool = ctx.enter_context(tc.tile_pool(name="v", bufs=2))
    epool = ctx.enter_context(tc.tile_pool(name="e", bufs=4))
    stpool = ctx.enter_context(tc.tile_pool(name="st", bufs=3))
    cpool = ctx.enter_context(tc.tile_pool(name="ctx", bufs=1))
    opool = ctx.enter_context(tc.tile_pool(name="o", bufs=1))
    ps_big = ctx.enter_context(tc.tile_pool(name="ps_big", bufs=2, space="PSUM"))
    ps_sc = ctx.enter_context(tc.tile_pool(name="ps_sc", bufs=2, space="PSUM"))
    ps_c = ctx.enter_context(tc.tile_pool(name="ps_c", bufs=2, space="PSUM"))

    w_tiles = {}

    def load_weights():
        for name, ap in (("wq", wq_ap), ("wk", wk_ap), ("wv", wv_ap), ("wo", wo_ap)):
            t = wpool.tile([128, HC, H], F32R, tag=name)
            nc.gpsimd.dma_start(t[:], ap.rearrange("(kc p) j -> p kc j", p=128))
            w_tiles[name] = t

    for m in range(M):
        # ---- Phase A: load x tiles for this modality, transpose to xT ----
        xt = xtp.tile([128, HC, PM], F32R, tag="xt")
        xtiles = []
        for ti in range(TCM):
            xtile = xin.tile([128, H], F32R, tag="xin")
            row0 = (m * TCM + ti) * 128
            nc.gpsimd.dma_start(xtile[:], x_ap[row0:row0 + 128, :])
            xtiles.append(xtile)
        if m == 0:
            # Emit weight DMAs after the first x tiles so the transposes can
            # start while the (much larger) weight transfers stream in.
            load_weights()
        for ti in range(TCM):
            for hc in range(HC):
                pst = ps_big.tile([128, 512], F32R, tag="ps_big")
                nc.tensor.transpose(
                    pst[:, :128], xtiles[ti][:, hc * 128:(hc + 1) * 128], identity[:]
                )
                nc.vector.tensor_copy(
                    xt[:, hc, ti * 128:(ti + 1) * 128], pst[:, :128]
                )

        # ---- Phase B: projections ----
        qt = qpool.tile([128, HC, PM], F32R, tag="q")
        kt = kpool.tile([128, HC, PM], F32R, tag="k")
        for dst, w in ((qt, w_tiles["wq"]), (kt, w_tiles["wk"])):
            for jc in range(HC):
                ps = ps_big.tile([128, 512], F32, tag="ps_big")
                for kc in range(HC):
                    nc.tensor.matmul(
                        ps[:],
                        w[:, kc, jc * 128:(jc + 1) * 128],
                        xt[:, kc, :],
                        start=(kc == 0),
                        stop=(kc == HC - 1),
                    )
                nc.vector.tensor_copy(dst[:, jc, :], ps[:])

        vt = vpool.tile([128, TCM, NH, HS + 1], F32R, tag="v")
        nc.vector.tensor_copy(
            vt[:, :, :, HS],
            onescol[:].rearrange("p (t h) -> p t h", t=TCM),
        )
        for ti in range(TCM):
            for nn in range(2):
                ps = ps_big.tile([128, 512], F32, tag="ps_big")
                for kc in range(HC):
                    nc.tensor.matmul(
                        ps[:, :384],
                        xt[:, kc, ti * 128:(ti + 1) * 128],
                        w_tiles["wv"][:, kc, nn * 384:(nn + 1) * 384],
                        start=(kc == 0),
                        stop=(kc == HC - 1),
                    )
                nc.vector.tensor_copy(
                    vt[:, ti, nn * 6:(nn + 1) * 6, :HS],
                    ps[:, :384].rearrange("p (h c) -> p h c", c=HS),
                )

        # ---- Phase C: attention per head ----
        # eT = exp(scoresT/8) in two [128,1024] halves (double-buffered
        # 2-bank PSUM so ScalarE overlaps the next scores matmuls).
        # Softmax sums ride along as the v_aug ones-column -> psc row 64;
        # reciprocal_approx_fast + K=1 ones outer-product broadcasts 1/s,
        # then the unnormalized ctx (already evacuated) is scaled in place.
        ctxt = cpool.tile([128, HC, PM], F32R, tag="ctx")
        for h in range(NH):
            hc, hr = h // 2, (h % 2) * 64
            qh = qt[hr:hr + 64, hc, :]
            ets = []
            for half in range(2):
                pssc = ps_sc.tile([128, 1024], F32, tag="ps_sc")
                for j2 in range(2):
                    jc = half * 2 + j2
                    nc.tensor.matmul(
                        pssc[:, j2 * 512:(j2 + 1) * 512],
                        kt[hr:hr + 64, hc, jc * 128:(jc + 1) * 128],
                        qh,
                        start=True,
                        stop=True,
                    )
                et = epool.tile([128, 1024], F32R, tag="e")
                nc.scalar.activation(et[:], pssc[:], AF.Exp, scale=0.125)
                ets.append(et)
            psc = ps_c.tile([HS + 1, 512], F32, tag="ps_c")
            for jc in range(TCM):
                nc.tensor.matmul(
                    psc[:],
                    vt[:, jc, h, :],
                    ets[jc // 2][:, (jc % 2) * 512:(jc % 2 + 1) * 512],
                    start=(jc == 0),
                    stop=(jc == TCM - 1),
                )
            nc.vector.tensor_copy(ctxt[hr:hr + 64, hc, :], psc[:HS, :])
            rf = stpool.tile([1, 512], F32, tag="rf")
            nc.vector.reciprocal_approx_fast(out=rf[:], in_=psc[HS:HS + 1, :])
            rfr = stpool.tile([1, 512], F32R, tag="rfr")
            nc.scalar.activation(rfr[:], rf[:], AF.Copy)
            psb = ps_big.tile([128, 512], F32, tag="ps_big")
            nc.tensor.matmul(
                psb[:64, :], ones64[:], rfr[:], start=True, stop=True
            )
            nc.vector.tensor_tensor(
                ctxt[hr:hr + 64, hc, :], ctxt[hr:hr + 64, hc, :],
                psb[:64, :], ALU.mult,
            )

        # ---- Phase D: output projection ----
        for ti in range(TCM):
            osb = opool.tile([128, H], F32, tag="o")
            for nn in range(2):
                ps = ps_big.tile([128, 512], F32, tag="ps_big")
                for cc in range(HC):
                    nc.tensor.matmul(
                        ps[:, :384],
                        ctxt[:, cc, ti * 128:(ti + 1) * 128],
                        w_tiles["wo"][:, cc, nn * 384:(nn + 1) * 384],
                        start=(cc == 0),
                        stop=(cc == HC - 1),
                    )
                nc.vector.tensor_copy(osb[:, nn * 384:(nn + 1) * 384], ps[:, :384])
            row0 = (m * TCM + ti) * 128
            nc.sync.dma_start(out_ap[row0:row0 + 128, :], osb[:])


_NC_CACHE = {}


def build_nc():
    if "nc" not in _NC_CACHE:
        nc = bacc.Bacc("TRN2", target_bir_lowering=False, debug=False, num_devices=B)
        with TileContext(nc) as tc:
            with ExitStack() as stack:
                _emit(tc, stack)
        nc.compile()
        _NC_CACHE["nc"] = nc
    return _NC_CACHE["nc"]


def _numpy_fallback(x, Wq, bq, Wk, bk, Wv, bv, Wo, bo):
    Bb, Mm, Pp, Hh = x.shape
    xx = x.reshape(-1, Hh)
    q = (xx @ Wq + bq).reshape(Bb, Mm, Pp, NH, HS).transpose(0, 1, 3, 2, 4)
    k = (xx @ Wk + bk).reshape(Bb, Mm, Pp, NH, HS).transpose(0, 1, 3, 2, 4)
    v = (xx @ Wv + bv).reshape(Bb, Mm, Pp, NH, HS).transpose(0, 1, 3, 2, 4)
    s = np.einsum("bmnqh,bmnkh->bmnqk", q, k) / np.sqrt(HS)
    s = s - s.max(axis=-1, keepdims=True)
    e = np.exp(s)
    p = e / e.sum(axis=-1, keepdims=True)
    ctx = np.einsum("bmnqk,bmnkh->bmnqh", p, v)
    ctx = ctx.transpose(0, 1, 3, 2, 4).reshape(Bb, Mm, Pp, Hh)
    return (ctx @ Wo + bo).astype(np.float32)


def kernel(hidden_states, Wq, bq, Wk, bk, Wv, bv, Wo, bo):
    hs = np.ascontiguousarray(np.asarray(hidden_states, dtype=np.float32))
    ws = {n: np.ascontiguousarray(np.asarray(w, dtype=np.float32))
          for n, w in (("wq", Wq), ("wk", Wk), ("wv", Wv), ("wo", Wo))}
    biases = [np.asarray(b, dtype=np.float32) for b in (bq, bk, bv, bo)]
    if any(np.any(b) for b in biases):
        return _numpy_fallback(hs, ws["wq"], biases[0], ws["wk"], biases[1],
                               ws["wv"], biases[2], ws["wo"], biases[3])

    nc = build_nc()
    in_maps = [
        {"x": hs[b].reshape(T, H), **ws}
        for b in range(B)
    ]
    res = bass_utils.run_bass_kernel_spmd(nc, in_maps, core_ids=list(range(B)))
    out = np.stack([res.results[b]["out"].reshape(M, PM, H) for b in range(B)])
    return out.astype(np.float32)


# revision 30
# speedup vs baseline: 1.6524x; 1.1615x over previous
"""TRN2 Bass kernel for nn_Attention_m_17815524344494.

Multi-head attention over [B=8, M=4, P=512, H=768], nh=12, hs=64.
Sharding: data-parallel over batch B -> one batch element per NeuronCore (8 cores).

Per-core dataflow (T = M*P = 2048 tokens, all matmuls in float32r ~ TF32):
  1. xT [768,2048] (pre-transposed on host) DMA'd feature-major per modality
  2. qT = Wq^T xT, kT = Wk^T xT (feature-major), v = x Wv (token-major,
     augmented with a ones column per head for free softmax sums)
  3. per (modality, head): scoresT = kT^T q (keys on partitions),
     eT = exp(scoresT/8) via ScalarE, ctxT_unnorm/sums = v_aug^T eT,
     sums broadcast via K=1 outer-product matmul, normalize on VectorE
  4. out = ctxT^T Wo (token-major), DMA to DRAM

Biases are zeros per the problem spec; a numpy fallback handles the
(never exercised) nonzero-bias case.
"""

from contextlib import ExitStack

import numpy as np

import concourse.bass as bass
import concourse.mybir as mybir
from concourse import bacc, bass_utils
from concourse.tile import TileContext
from concourse.masks import make_identity

F32 = mybir.dt.float32
F32R = mybir.dt.float32r
AF = mybir.ActivationFunctionType
ALU = mybir.AluOpType

B, M, PM, H = 8, 4, 512, 768
NH, HS = 12, 64
T = M * PM          # 2048 tokens per core
HC = H // 128       # 6 hidden chunks
TCM = PM // 128     # 4 token chunks per modality


def _emit(tc, ctx):
    nc = tc.nc

    x_ap = nc.dram_tensor("x", [H, T], F32, kind="ExternalInput").ap()
    wq_ap = nc.dram_tensor("wq", [H, H], F32, kind="ExternalInput").ap()
    wk_ap = nc.dram_tensor("wk", [H, H], F32, kind="ExternalInput").ap()
    wv_ap = nc.dram_tensor("wv", [H, H], F32, kind="ExternalInput").ap()
    wo_ap = nc.dram_tensor("wo", [H, H], F32, kind="ExternalInput").ap()
    out_ap = nc.dram_tensor("out", [T, H], F32, kind="ExternalOutput").ap()

    const = ctx.enter_context(tc.tile_pool(name="const", bufs=1))

    # f32r tiles can't be written by memset/affine_select directly (no
    # f32r rounding on those ISA paths); stage in f32 and copy via DVE.
    onescol = const.tile([128, NH * TCM], F32R)
    ones64 = const.tile([1, 64], F32R)
    with tc.tile_pool(name="stage", bufs=1) as stage:
        ones_stage = stage.tile([128, 64], F32)
        nc.gpsimd.memset(ones_stage[:], 1.0)
        nc.vector.tensor_copy(onescol[:], ones_stage[:, :NH * TCM])
        nc.vector.tensor_copy(ones64[:], ones_stage[:1, :])

    wpool = ctx.enter_context(tc.tile_pool(name="w", bufs=1))
    xtp = ctx.enter_context(tc.tile_pool(name="xt", bufs=1))
    qpool = ctx.enter_context(tc.tile_pool(name="q", bufs=2))
    kpool = ctx.enter_context(tc.tile_pool(name="k", bufs=2))
    vpool = ctx.enter_context(tc.tile_pool(name="v", bufs=2))
    epool = ctx.enter_context(tc.tile_pool(name="e", bufs=4))
    stpool = ctx.enter_context(tc.tile_pool(name="st", bufs=3))
    rfrpool = ctx.enter_context(tc.tile_pool(name="rfr", bufs=7))
    cpool = ctx.enter_context(tc.tile_pool(name="ctx", bufs=1))
    opool = ctx.enter_context(tc.tile_pool(name="o", bufs=1))
    ps_big = ctx.enter_context(tc.tile_pool(name="ps_big", bufs=2, space="PSUM"))
    ps_sc = ctx.enter_context(tc.tile_pool(name="ps_sc", bufs=2, space="PSUM"))
    ps_c = ctx.enter_context(tc.tile_pool(name="ps_c", bufs=2, space="PSUM"))

    w_tiles = {}

    def load_weights():
        for name, ap in (("wq", wq_ap), ("wk", wk_ap), ("wv", wv_ap), ("wo", wo_ap)):
            t = wpool.tile([128, HC, H], F32R, tag=name)
            nc.gpsimd.dma_start(t[:], ap.rearrange("(kc p) j -> p kc j", p=128))
            w_tiles[name] = t

    for m in range(M):
        # ---- Phase A: DMA this modality's slice of the pre-transposed x ----
        xt = xtp.tile([128, HC, PM], F32R, tag="xt")
        nc.gpsimd.dma_start(
            xt[:],
            x_ap.rearrange("(hc p) t -> p hc t", p=128)[:, :, m * PM:(m + 1) * PM],
        )
        if m == 0:
            load_weights()

        # ---- Phase B: projections ----
        qt = qpool.tile([128, HC, PM], F32R, tag="q")
        kt = kpool.tile([128, HC, PM], F32R, tag="k")
        for dst, w in ((qt, w_tiles["wq"]), (kt, w_tiles["wk"])):
            for jc in range(HC):
                ps = ps_big.tile([128, 512], F32, tag="ps_big")
                for kc in range(HC):
                    nc.tensor.matmul(
                        ps[:],
                        w[:, kc, jc * 128:(jc + 1) * 128],
                        xt[:, kc, :],
                        start=(kc == 0),
                        stop=(kc == HC - 1),
                    )
                nc.vector.tensor_copy(dst[:, jc, :], ps[:])

        vt = vpool.tile([128, TCM, NH, HS + 1], F32R, tag="v")
        nc.vector.tensor_copy(
            vt[:, :, :, HS],
            onescol[:].rearrange("p (t h) -> p t h", t=TCM),
        )
        for ti in range(TCM):
            for nn in range(2):
                ps = ps_big.tile([128, 512], F32, tag="ps_big")
                for kc in range(HC):
                    nc.tensor.matmul(
                        ps[:, :384],
                        xt[:, kc, ti * 128:(ti + 1) * 128],
                        w_tiles["wv"][:, kc, nn * 384:(nn + 1) * 384],
                        start=(kc == 0),
                        stop=(kc == HC - 1),
                    )
                nc.vector.tensor_copy(
                    vt[:, ti, nn * 6:(nn + 1) * 6, :HS],
                    ps[:, :384].rearrange("p (h c) -> p h c", c=HS),
                )

        # ---- Phase C: attention per head ----
        # eT = exp(scoresT/8) in two [128,1024] halves (double-buffered
        # 2-bank PSUM so ScalarE overlaps the next scores matmuls).
        # Softmax sums ride along as the v_aug ones-column -> psc row 64;
        # reciprocal_approx_fast + K=1 ones outer-product broadcasts 1/s,
        # then the unnormalized ctx (already evacuated) is scaled in place.
        ctxt = cpool.tile([128, HC, PM], F32R, tag="ctx")
        rfrs = []

        def normalize(heads):
            # Broadcast each head's 1/sum via K=1 outer product and scale the
            # unnormalized ctx in place; batched so the PE never stalls
            # mid-stream waiting on the recip chain.
            for h in heads:
                hc, hr = h // 2, (h % 2) * 64
                psb = ps_big.tile([128, 512], F32, tag="ps_big")
                nc.tensor.matmul(
                    psb[:64, :], ones64[:], rfrs[h][:], start=True, stop=True
                )
                nc.vector.tensor_tensor(
                    ctxt[hr:hr + 64, hc, :], ctxt[hr:hr + 64, hc, :],
                    psb[:64, :], ALU.mult,
                )

        for h in range(NH):
            hc, hr = h // 2, (h % 2) * 64
            qh = qt[hr:hr + 64, hc, :]
            ets = []
            for half in range(2):
                pssc = ps_sc.tile([128, 1024], F32, tag="ps_sc")
                for j2 in range(2):
                    jc = half * 2 + j2
                    nc.tensor.matmul(
                        pssc[:, j2 * 512:(j2 + 1) * 512],
                        kt[hr:hr + 64, hc, jc * 128:(jc + 1) * 128],
                        qh,
                        start=True,
                        stop=True,
                    )
                et = epool.tile([128, 1024], F32R, tag="e")
                nc.scalar.activation(et[:], pssc[:], AF.Exp, scale=0.125)
                ets.append(et)
            psc = ps_c.tile([HS + 1, 512], F32, tag="ps_c")
            for jc in range(TCM):
                nc.tensor.matmul(
                    psc[:],
                    vt[:, jc, h, :],
                    ets[jc // 2][:, (jc % 2) * 512:(jc % 2 + 1) * 512],
                    start=(jc == 0),
                    stop=(jc == TCM - 1),
                )
            nc.vector.tensor_copy(ctxt[hr:hr + 64, hc, :], psc[:HS, :])
            rf = stpool.tile([1, 512], F32, tag="rf")
            nc.vector.reciprocal_approx_fast(out=rf[:], in_=psc[HS:HS + 1, :])
            rfr = rfrpool.tile([1, 512], F32R, tag="rfr")
            nc.scalar.activation(rfr[:], rf[:], AF.Copy)
            rfrs.append(rfr)
            if h == 5:
                normalize(range(0, 6))
        normalize(range(6, NH))

        # ---- Phase D: output projection ----
        for ti in range(TCM):
            osb = opool.tile([128, H], F32, tag="o")
            for nn in range(2):
                ps = ps_big.tile([128, 512], F32, tag="ps_big")
                for cc in range(HC):
                    nc.tensor.matmul(
                        ps[:, :384],
                        ctxt[:, cc, ti * 128:(ti + 1) * 128],
                        w_tiles["wo"][:, cc, nn * 384:(nn + 1) * 384],
                        start=(cc == 0),
                        stop=(cc == HC - 1),
                    )
                nc.vector.tensor_copy(osb[:, nn * 384:(nn + 1) * 384], ps[:, :384])
            row0 = (m * TCM + ti) * 128
            nc.sync.dma_start(out_ap[row0:row0 + 128, :], osb[:])


_NC_CACHE = {}


def build_nc():
    if "nc" not in _NC_CACHE:
        nc = bacc.Bacc("TRN2", target_bir_lowering=False, debug=False, num_devices=B)
        with TileContext(nc) as tc:
            with ExitStack() as stack:
                _emit(tc, stack)
        nc.compile()
        _NC_CACHE["nc"] = nc
    return _NC_CACHE["nc"]


def _numpy_fallback(x, Wq, bq, Wk, bk, Wv, bv, Wo, bo):
    Bb, Mm, Pp, Hh = x.shape
    xx = x.reshape(-1, Hh)
    q = (xx @ Wq + bq).reshape(Bb, Mm, Pp, NH, HS).transpose(0, 1, 3, 2, 4)
    k = (xx @ Wk + bk).reshape(Bb, Mm, Pp, NH, HS).transpose(0, 1, 3, 2, 4)
    v = (xx @ Wv + bv).reshape(Bb, Mm, Pp, NH, HS).transpose(0, 1, 3, 2, 4)
    s = np.einsum("bmnqh,bmnkh->bmnqk", q, k) / np.sqrt(HS)
    s = s - s.max(axis=-1, keepdims=True)
    e = np.exp(s)
    p = e / e.sum(axis=-1, keepdims=True)
    ctx = np.einsum("bmnqk,bmnkh->bmnqh", p, v)
    ctx = ctx.transpose(0, 1, 3, 2, 4).reshape(Bb, Mm, Pp, Hh)
    return (ctx @ Wo + bo).astype(np.float32)


def kernel(hidden_states, Wq, bq, Wk, bk, Wv, bv, Wo, bo):
    hs = np.ascontiguousarray(np.asarray(hidden_states, dtype=np.float32))
    ws = {n: np.ascontiguousarray(np.asarray(w, dtype=np.float32))
          for n, w in (("wq", Wq), ("wk", Wk), ("wv", Wv), ("wo", Wo))}
    biases = [np.asarray(b, dtype=np.float32) for b in (bq, bk, bv, bo)]
    if any(np.any(b) for b in biases):
        return _numpy_fallback(hs, ws["wq"], biases[0], ws["wk"], biases[1],
                               ws["wv"], biases[2], ws["wo"], biases[3])

    nc = build_nc()
    in_maps = [
        {"x": np.ascontiguousarray(hs[b].reshape(T, H).T), **ws}
        for b in range(B)
    ]
    res = bass_utils.run_bass_kernel_spmd(nc, in_maps, core_ids=list(range(B)))
    out = np.stack([res.results[b]["out"].reshape(M, PM, H) for b in range(B)])
    return out.astype(np.float32)
